# revision 11
# baseline (speedup 1.0000x reference)
"""Trainium2 Bass kernel for nn_CrossAttention_DenseAVInteractions.

Math: the reference builds a cartesian KV grid kv[b,i,j] = pv[b,i] + pa[b,j]
over (N_v, N_a) and attends 64 queries against all N_v*N_a = 65536 keys.
Because the logits decompose as s[q,(i,j)] = (q.k_v[i]) + (q.k_a[j]), the
softmax over the product grid factorizes exactly:

    p[q,(i,j)] = softmax_i(q.k_v)[q,i] * softmax_j(q.k_a)[q,j]
    out[q]     = softmax_i(q.k_v) @ v_v + softmax_j(q.k_a) @ v_a

so the whole attention reduces to two 256-key attentions per (b, h).

Sharding (8 cores): core c handles batch b = c // 4 and the head pair
(2j, 2j+1) with j = c % 4.  Each core computes its heads' partial output
projection partial = out_heads @ Wproj[:, head_cols].T in f32; the host sums
the 4 partials per batch and adds bproj.

Device-side layout choices (from the first profile round):
 - All per-core inputs are packed on the host into ONE [128, 5376] f32
   tensor (contraction dim on partitions everywhere), loaded with 6 large
   DMAs split between the two HWDGE engines (sync + scalar) — per-DMA
   trigger cost is ~0.6us regardless of size, so few big transfers win.
 - All matmuls run with operands bitcast to float32r: same f32 bits, but
   1 cycle/row at free-dim >= 256 instead of fp32's two half-rate passes.
 - Softmax skips max-subtraction (logits here are ~N(0, 0.2^2), exp is
   safe) and uses the scalar engine's Exp activation with accum_out to
   get the denominator for free.
"""

import os
import sys

import numpy as np

sys.path.insert(0, "/opt/trn_rl_repo")

DIM = 512
H = 8
HD = DIM // H          # 64
B = 2
N_MM = 64
N_A = 256
N_V = 256
SCALE = HD ** -0.5     # 0.125
N_CORES = 8

# packA column offsets (f32 columns in the [128, 5376] packed input)
O_WQ = 0          # 4 k-tiles x 128
O_XMM = 512       # 4 x 64
O_WKV = 768       # 4 x 128
O_XV = 1280       # 4 x 256
O_WKA = 2304      # 4 x 128
O_XA = 2816       # 4 x 256
O_WVV = 3840      # 4 x 128
O_WVA = 4352      # 4 x 128
O_WPROJ = 4864    # [128ch, 512]
PACK_COLS = 5376

# chunk boundaries (cols) and which engine issues the load
CHUNKS = [
    (0, 768, "sync"),        # wq + xmm
    (768, 1792, "scalar"),   # wkv + xv k0,k1
    (1792, 2816, "sync"),    # xv k2,k3 + wka
    (2816, 3840, "scalar"),  # xa
    (3840, 4864, "sync"),    # wvv + wva
    (4864, 5376, "scalar"),  # wproj
]

_cached = {}


def _build_program():
    import concourse.bacc as bacc
    from concourse import mybir
    from concourse.tile import TileContext

    f32 = mybir.dt.float32
    f32r = mybir.dt.float32r
    nc = bacc.Bacc(name="cross_attn_dense_av")

    packA = nc.dram_tensor("packA", [128, PACK_COLS], f32, kind="ExternalInput")
    out_d = nc.dram_tensor("out", [64, 512], f32, kind="ExternalOutput")
    ident_d = nc.inline_tensor(np.eye(128, dtype=np.float32), name="ident128")
    ident2_d = nc.inline_tensor(
        np.tile(np.eye(64, dtype=np.float32), (2, 1)), name="ident64x2"
    )

    from contextlib import ExitStack

    def r(ap):
        return ap  # fp32 everywhere: f32r measured at ~1.5e-4 rel err (too lossy)

    with TileContext(nc) as tc, ExitStack() as ctx:
        io = ctx.enter_context(tc.tile_pool(name="io", bufs=1))
        work = ctx.enter_context(tc.tile_pool(name="work", bufs=1))
        ps_mm = ctx.enter_context(tc.tile_pool(name="ps_mm", bufs=3, space="PSUM"))
        ps_s = ctx.enter_context(tc.tile_pool(name="ps_s", bufs=1, space="PSUM"))
        ps_pt = ctx.enter_context(tc.tile_pool(name="ps_pt", bufs=2, space="PSUM"))
        ps_o = ctx.enter_context(tc.tile_pool(name="ps_o", bufs=1, space="PSUM"))
        ps_f = ctx.enter_context(tc.tile_pool(name="ps_f", bufs=1, space="PSUM"))

        # ---- loads: few large DMAs, two HWDGE engines in parallel ----
        chunk_t = {}
        for lo, hi, eng in CHUNKS:
            t = io.tile([128, hi - lo], f32, tag=f"c{lo}")
            getattr(nc, eng).dma_start(out=t, in_=packA[:, lo:hi])
            chunk_t[lo] = t
        ident = io.tile([128, 128], f32, tag="ident")
        nc.scalar.dma_start(out=ident, in_=ident_d[:, :])
        ident2 = io.tile([128, 64], f32, tag="ident2")
        nc.sync.dma_start(out=ident2, in_=ident2_d[:, :])

        # ---- PE warmup: ~8 bf16 matmuls on memset scratch trip the HAM
        #      clock gate to 8/8 while the input DMAs are still in flight ----
        bf16 = mybir.dt.bfloat16
        warm_sb = io.tile([128, 512], bf16, tag="warm_sb")
        nc.vector.memset(warm_sb, 0.5)
        warm_ps = ps_f.tile([128, 512], f32, tag="f_ps")
        for w in range(8):
            nc.tensor.matmul(
                warm_ps, warm_sb[:, 0:128], warm_sb,
                start=(w == 0), stop=(w == 7),
            )

        def col(off, width):
            """AP slice of the packed input at absolute column offset."""
            for lo, hi, _ in CHUNKS:
                if lo <= off and off + width <= hi:
                    return chunk_t[lo][:, off - lo:off - lo + width]
            raise ValueError(f"span {off}:{off + width} crosses chunk boundary")

        # ---- projections (channels on partitions for q/k; then v gets
        #      PE-transposed to tokens-on-partitions) ----
        # q2T [128ch, 64q], scaled by SCALE on evacuation
        q_ps = ps_mm.tile([128, 64], f32, tag="mm")
        for k in range(4):
            nc.tensor.matmul(
                q_ps, r(col(O_WQ + 128 * k, 128)), r(col(O_XMM + 64 * k, 64)),
                start=(k == 0), stop=(k == 3),
            )
        q2T = work.tile([128, 64], f32, tag="q2T")
        nc.scalar.mul(q2T, q_ps, SCALE)

        k_sb, v_sb = [], []
        for side, (o_wk, o_wv, o_x) in enumerate(
            [(O_WKV, O_WVV, O_XV), (O_WKA, O_WVA, O_XA)]
        ):
            # kT [128ch, 256tok]
            kp = ps_mm.tile([128, 256], f32, tag="mm")
            for k in range(4):
                nc.tensor.matmul(
                    kp, r(col(o_wk + 128 * k, 128)), r(col(o_x + 256 * k, 256)),
                    start=(k == 0), stop=(k == 3),
                )
            ks = work.tile([128, 256], f32, tag=f"k_sb{side}")
            nc.vector.tensor_copy(ks, kp)
            k_sb.append(ks)

            # vT [128ch, 256tok], then PE-transpose to v [2tok-tiles, 128, 128ch]
            vp = ps_mm.tile([128, 256], f32, tag="mm")
            for k in range(4):
                nc.tensor.matmul(
                    vp, r(col(o_wv + 128 * k, 128)), r(col(o_x + 256 * k, 256)),
                    start=(k == 0), stop=(k == 3),
                )
            vTs = work.tile([128, 256], f32, tag=f"vT_sb{side}")
            nc.scalar.copy(vTs, vp)
            vt_ps = ps_mm.tile([128, 256], f32, tag="mm")
            for t in range(2):
                nc.tensor.transpose(
                    r(vt_ps[:, 128 * t:128 * t + 128]),
                    r(vTs[:, 128 * t:128 * t + 128]),
                    r(ident),
                )
            vs = work.tile([128, 2, 128], f32, tag=f"v_sb{side}")
            nc.vector.tensor_copy(vs, vt_ps.rearrange("p (t c) -> p t c", t=2))
            v_sb.append(vs)

        # ---- scores: partitions = 64*h + q, free = (side, keys); the two
        #      heads run in the (0,0)/(64,64) array quadrants concurrently.
        #      Heads-on-partitions lets each side's exp run as ONE
        #      full-128-partition activation with a fused accumulator. ----
        s_ps = ps_s.tile([128, 2, 256], f32, tag="s")
        for h in range(2):
            hs = slice(64 * h, 64 * h + 64)
            for side in range(2):
                nc.tensor.matmul(
                    s_ps[hs, side, :], q2T[hs, :], k_sb[side][hs, :],
                    start=True, stop=True,
                    tile_position=(64 * h, 64 * h),
                )
        # softmax over keys (no max-subtraction: |s| < ~2 by construction)
        p_all = work.tile([128, 2, 256], f32, tag="p_all")
        zsum = work.tile([128, 2], f32, tag="zsum")
        zrec = work.tile([128, 2], f32, tag="zrec")
        for side in range(2):
            nc.scalar.activation(
                p_all[:, side, :], s_ps[:, side, :],
                mybir.ActivationFunctionType.Exp,
                accum_out=zsum[:, side:side + 1],
            )
            nc.vector.reciprocal(zrec[:, side:side + 1], zsum[:, side:side + 1])
            nc.vector.tensor_scalar_mul(
                p_all[:, side, :], p_all[:, side, :], zrec[:, side:side + 1]
            )
        # transpose p -> [keys, q] (two 128-key blocks per (h, side))
        pt_all = []
        for h in range(2):
            hs = slice(64 * h, 64 * h + 64)
            pt_h = []
            for side in range(2):
                pt_ps = ps_pt.tile([128, 128], f32, tag="pt")
                for t in range(2):
                    nc.tensor.transpose(
                        pt_ps[:, 64 * t:64 * t + 64],
                        p_all[hs, side, 128 * t:128 * t + 128],
                        ident2[hs, :],
                    )
                pt = work.tile([128, 128], f32, tag=f"pt_sb{h}{side}")
                (nc.vector.tensor_copy if side == 0 else nc.scalar.copy)(pt, pt_ps)
                pt_h.append(pt)
            pt_all.append(pt_h)

        # PV: o[128ch(2 heads), 64q] accumulated per head (col-tiled for h=1)
        o_ps = ps_o.tile([128, 64], f32, tag="o")
        for h in range(2):
            hs = slice(64 * h, 64 * h + 64)
            n = 0
            for side in range(2):
                for t in range(2):
                    nc.tensor.matmul(
                        o_ps[hs, :],
                        r(v_sb[side][:, t, hs]),
                        r(pt_all[h][side][:, 64 * t:64 * t + 64]),
                        start=(n == 0), stop=(n == 3),
                        tile_position=(0, 64 * h),
                    )
                    n += 1
        o_sb = work.tile([128, 64], f32, tag="o_sb")
        nc.scalar.copy(o_sb, o_ps)

        # output projection partial: [64q, 512]
        f_ps = ps_f.tile([64, 512], f32, tag="f_ps")
        nc.tensor.matmul(f_ps, r(o_sb), r(col(O_WPROJ, 512)), start=True, stop=True)
        f_sb = work.tile([64, 512], f32, tag="f_sb")
        nc.vector.tensor_copy(f_sb[:, 0:256], f_ps[:, 0:256])
        nc.scalar.copy(f_sb[:, 256:512], f_ps[:, 256:512])
        nc.sync.dma_start(out=out_d[:, :], in_=f_sb)

    nc.finalize()
    return nc


def _km(a):
    """[512, C] K-major -> [128, 4*C] (4 k-tiles side by side)."""
    c = a.shape[1]
    return a.reshape(4, 128, c).transpose(1, 0, 2).reshape(128, 4 * c)


def _shard_inputs(xmm, xa, xv, Wq, Wkv, Wproj):
    """Build the 8 per-core input maps (one packed [128, 5376] tensor each)."""
    in_maps = []
    for core in range(N_CORES):
        b, j = divmod(core, 4)
        r = slice(128 * j, 128 * j + 128)               # head-pair rows in [0,512)
        rv = slice(512 + 128 * j, 512 + 128 * j + 128)  # v rows in Wkv
        pack = np.concatenate(
            [
                _km(Wq[r, :].T),            # O_WQ
                _km(xmm[b].T),              # O_XMM
                _km(Wkv[r, :512].T),        # O_WKV
                _km(xv[b].T),               # O_XV
                _km(Wkv[r, 512:].T),        # O_WKA
                _km(xa[b].T),               # O_XA
                _km(Wkv[rv, :512].T),       # O_WVV
                _km(Wkv[rv, 512:].T),       # O_WVA
                Wproj[:, 128 * j:128 * j + 128].T,  # O_WPROJ
            ],
            axis=1,
        )
        assert pack.shape == (128, PACK_COLS)
        in_maps.append({"packA": np.ascontiguousarray(pack, np.float32)})
    return in_maps


def _get_program():
    if "nc" not in _cached:
        _cached["nc"] = _build_program()
    return _cached["nc"]


def _register_ntff_hook():
    """Best-effort: register the axon NTFF profile hook that the container's
    antenv stub doesn't provide, so run_bass_kernel_spmd(trace=True) can
    measure HW exec time. No-op on failure."""
    try:
        import types

        try:
            from antenv.axon_hooks import get_axon_ntff_profile_hook
            if get_axon_ntff_profile_hook() is not None:
                return
        except ImportError:
            pass
        import antenv
        from trn_agent_boot.trn_boot import _ntff_profile_via_ctypes

        hook = _ntff_profile_via_ctypes("/opt/axon/libaxon_pjrt.so")
        mod = types.ModuleType("antenv.axon_hooks")
        mod._hook = hook
        mod.set_axon_ntff_profile_hook = lambda h: setattr(mod, "_hook", h)
        mod.get_axon_ntff_profile_hook = lambda: mod._hook
        sys.modules["antenv.axon_hooks"] = mod
        antenv.axon_hooks = mod

        # artifact upload has no backing store in this container
        from concourse import bass_utils

        bass_utils.upload_artifacts = lambda tmpdir: tmpdir
    except Exception as e:  # pragma: no cover
        print(f"ntff hook registration failed: {e}", file=sys.stderr)


def kernel(xmm, xa, xv, Wq, Wkv, Wproj, bproj, _want_profile=False):
    from concourse.bass_utils import run_bass_kernel_spmd

    if _want_profile:
        _register_ntff_hook()
    nc = _get_program()
    in_maps = _shard_inputs(
        np.asarray(xmm, np.float32), np.asarray(xa, np.float32),
        np.asarray(xv, np.float32), np.asarray(Wq, np.float32),
        np.asarray(Wkv, np.float32), np.asarray(Wproj, np.float32),
    )
    res = run_bass_kernel_spmd(
        nc, in_maps, core_ids=list(range(N_CORES)), trace=_want_profile
    )
    out = np.zeros((B, N_MM, DIM), np.float32)
    for core in range(N_CORES):
        out[core // 4] += res.results[core]["out"]
    out += np.asarray(bproj, np.float32)[None, None, :]
    if _want_profile:
        return out, res
    return out


# revision 14
# speedup vs baseline: 1.1953x; 1.1953x over previous
"""Trainium2 Bass kernel for nn_CrossAttention_DenseAVInteractions.

Math: the reference builds a cartesian KV grid kv[b,i,j] = pv[b,i] + pa[b,j]
over (N_v, N_a) and attends 64 queries against all N_v*N_a = 65536 keys.
Because the logits decompose as s[q,(i,j)] = (q.k_v[i]) + (q.k_a[j]), the
softmax over the product grid factorizes exactly:

    p[q,(i,j)] = softmax_i(q.k_v)[q,i] * softmax_j(q.k_a)[q,j]
    out[q]     = softmax_i(q.k_v) @ v_v + softmax_j(q.k_a) @ v_a

so the whole attention reduces to two 256-key attentions per (b, h).

Sharding (8 cores): core c handles batch b = c // 4 and the head pair
(2j, 2j+1) with j = c % 4.  Each core computes its heads' partial output
projection partial = out_heads @ Wproj[:, head_cols].T in f32; the host sums
the 4 partials per batch and adds bproj.

Device-side layout choices (from the first profile round):
 - All per-core inputs are packed on the host into ONE [128, 5376] f32
   tensor (contraction dim on partitions everywhere), loaded with 6 large
   DMAs split between the two HWDGE engines (sync + scalar) — per-DMA
   trigger cost is ~0.6us regardless of size, so few big transfers win.
 - All matmuls run with operands bitcast to float32r: same f32 bits, but
   1 cycle/row at free-dim >= 256 instead of fp32's two half-rate passes.
 - Softmax skips max-subtraction (logits here are ~N(0, 0.2^2), exp is
   safe) and uses the scalar engine's Exp activation with accum_out to
   get the denominator for free.
"""

import os
import sys

import numpy as np

sys.path.insert(0, "/opt/trn_rl_repo")

DIM = 512
H = 8
HD = DIM // H          # 64
B = 2
N_MM = 64
N_A = 256
N_V = 256
SCALE = HD ** -0.5     # 0.125
N_CORES = 8

# packA column offsets (f32 columns in the [128, 5376] packed input).
# Each HWDGE queue streams its half in consumption order; the v-weights and
# wproj land last because their dependent work (vT -> transpose -> PV, and
# the final projection) is the shortest tail.
O_WKV = 0         # 4 k-tiles x 128
O_XV = 512        # 4 x 256
O_WVV = 1536      # 4 x 128
O_WPROJ = 2048    # [128ch, 512]
O_WQ = 2560       # 4 x 128
O_XMM = 3072      # 4 x 64
O_WKA = 3328      # 4 x 128
O_XA = 3840       # 4 x 256
O_WVA = 4864      # 4 x 128
PACK_COLS = 5376

# chunk boundaries (cols) and which engine issues the load
CHUNKS = [
    (0, 512, "sync"),        # wkv
    (2560, 3328, "scalar"),  # wq + xmm
    (512, 1024, "sync"),     # xv k0,k1
    (3328, 3840, "scalar"),  # wka
    (1024, 1536, "sync"),    # xv k2,k3
    (3840, 4352, "scalar"),  # xa k0,k1
    (1536, 2048, "sync"),    # wvv
    (4352, 4864, "scalar"),  # xa k2,k3
    (2048, 2560, "sync"),    # wproj
    (4864, 5376, "scalar"),  # wva
]

_cached = {}


def _build_program():
    import concourse.bacc as bacc
    from concourse import mybir
    from concourse.tile import TileContext

    f32 = mybir.dt.float32
    f32r = mybir.dt.float32r
    nc = bacc.Bacc(name="cross_attn_dense_av")

    packA = nc.dram_tensor("packA", [128, PACK_COLS], f32, kind="ExternalInput")
    out_d = nc.dram_tensor("out", [64, 512], f32, kind="ExternalOutput")
    ident_d = nc.inline_tensor(np.eye(128, dtype=np.float32), name="ident128")
    ident2_d = nc.inline_tensor(
        np.tile(np.eye(64, dtype=np.float32), (2, 1)), name="ident64x2"
    )

    from contextlib import ExitStack

    def r(ap):
        return ap  # fp32 everywhere: f32r measured at ~1.5e-4 rel err (too lossy)

    with TileContext(nc) as tc, ExitStack() as ctx:
        io = ctx.enter_context(tc.tile_pool(name="io", bufs=1))
        work = ctx.enter_context(tc.tile_pool(name="work", bufs=1))
        ps_mm = ctx.enter_context(tc.tile_pool(name="ps_mm", bufs=4, space="PSUM"))
        ps_spt = ctx.enter_context(tc.tile_pool(name="ps_spt", bufs=2, space="PSUM"))
        ps_o = ctx.enter_context(tc.tile_pool(name="ps_o", bufs=1, space="PSUM"))
        ps_f = ctx.enter_context(tc.tile_pool(name="ps_f", bufs=1, space="PSUM"))

        # ---- loads: few large DMAs, two HWDGE engines in parallel ----
        chunk_t = {}
        for lo, hi, eng in CHUNKS:
            t = io.tile([128, hi - lo], f32, tag=f"c{lo}")
            getattr(nc, eng).dma_start(out=t, in_=packA[:, lo:hi])
            chunk_t[lo] = t
        ident = io.tile([128, 128], f32, tag="ident")
        nc.gpsimd.dma_start(out=ident, in_=ident_d[:, :])
        ident2 = io.tile([128, 64], f32, tag="ident2")
        nc.gpsimd.dma_start(out=ident2, in_=ident2_d[:, :])

        # ---- PE warmup: ~8 bf16 matmuls on memset scratch trip the HAM
        #      clock gate to 8/8 while the input DMAs are still in flight ----
        bf16 = mybir.dt.bfloat16
        warm_sb = io.tile([128, 512], bf16, tag="warm_sb")
        nc.vector.memset(warm_sb, 0.5)
        warm_ps = ps_f.tile([128, 512], f32, tag="f_ps")
        for w in range(8):
            nc.tensor.matmul(
                warm_ps, warm_sb[:, 0:128], warm_sb,
                start=(w == 0), stop=(w == 7),
            )

        def col(off, width):
            """AP slice of the packed input at absolute column offset."""
            for lo, hi, _ in CHUNKS:
                if lo <= off and off + width <= hi:
                    return chunk_t[lo][:, off - lo:off - lo + width]
            raise ValueError(f"span {off}:{off + width} crosses chunk boundary")

        # ---- pipelined compute, emitted in expected execution order ----
        def kproj(o_wk, o_x, side):
            """kT [128ch, 256tok] = Wk_side @ x_side.T"""
            kp = ps_mm.tile([128, 256], f32, tag="mm")
            for k in range(4):
                nc.tensor.matmul(
                    kp, col(o_wk + 128 * k, 128), col(o_x + 256 * k, 256),
                    start=(k == 0), stop=(k == 3),
                )
            ks = work.tile([128, 256], f32, tag=f"k_sb{side}")
            nc.vector.tensor_copy(ks, kp)
            return ks

        def vproj(o_wv, o_x, side):
            """v [128tok x 2tiles, 128ch] via vT matmul + PE transpose"""
            vp = ps_mm.tile([128, 256], f32, tag="mm")
            for k in range(4):
                nc.tensor.matmul(
                    vp, col(o_wv + 128 * k, 128), col(o_x + 256 * k, 256),
                    start=(k == 0), stop=(k == 3),
                )
            vTs = work.tile([128, 256], f32, tag=f"vT_sb{side}")
            nc.scalar.copy(vTs, vp)
            vt_ps = ps_mm.tile([128, 256], f32, tag="mm")
            for t in range(2):
                nc.tensor.transpose(
                    vt_ps[:, 128 * t:128 * t + 128],
                    vTs[:, 128 * t:128 * t + 128],
                    ident,
                )
            vs = work.tile([128, 2, 128], f32, tag=f"v_sb{side}")
            nc.vector.tensor_copy(vs, vt_ps.rearrange("p (t c) -> p t c", t=2))
            return vs

        def scores_softmax(ks, side):
            """scores (partitions = 64*h + q) + one full-width exp/normalize;
            returns normalized p [128, 256] for this side."""
            sp = ps_spt.tile([128, 256], f32, tag="spt")
            for h in range(2):
                hs = slice(64 * h, 64 * h + 64)
                nc.tensor.matmul(
                    sp[hs, :], q2T[hs, :], ks[hs, :],
                    start=True, stop=True, tile_position=(64 * h, 64 * h),
                )
            # softmax over keys (no max-subtraction: |s| < ~2 by construction)
            p = work.tile([128, 256], f32, tag=f"p{side}")
            zsum = work.tile([128, 1], f32, tag=f"zsum{side}")
            zrec = work.tile([128, 1], f32, tag=f"zrec{side}")
            nc.scalar.activation(
                p, sp, mybir.ActivationFunctionType.Exp, accum_out=zsum
            )
            nc.vector.reciprocal(zrec, zsum)
            nc.vector.tensor_scalar_mul(p, p, zrec)
            return p

        def ptrans(p, side):
            """transpose p -> [keys, q]: per head one [128, 128] tile"""
            pt_h = []
            for h in range(2):
                hs = slice(64 * h, 64 * h + 64)
                pt_ps = ps_spt.tile([128, 128], f32, tag="spt")
                for t in range(2):
                    nc.tensor.transpose(
                        pt_ps[:, 64 * t:64 * t + 64],
                        p[hs, 128 * t:128 * t + 128],
                        ident2[hs, :],
                    )
                pt = work.tile([128, 128], f32, tag=f"pt_sb{h}{side}")
                (nc.vector.tensor_copy if h == 0 else nc.scalar.copy)(pt, pt_ps)
                pt_h.append(pt)
            return pt_h

        # v-side chain first (its data streams in first), a-side behind it
        k_v = kproj(O_WKV, O_XV, 0)

        q_ps = ps_mm.tile([128, 64], f32, tag="mm")
        for k in range(4):
            nc.tensor.matmul(
                q_ps, col(O_WQ + 128 * k, 128), col(O_XMM + 64 * k, 64),
                start=(k == 0), stop=(k == 3),
            )
        q2T = work.tile([128, 64], f32, tag="q2T")
        nc.scalar.mul(q2T, q_ps, SCALE)

        p_v = scores_softmax(k_v, 0)
        v_v = vproj(O_WVV, O_XV, 0)
        pt_v = ptrans(p_v, 0)

        k_a = kproj(O_WKA, O_XA, 1)
        p_a = scores_softmax(k_a, 1)
        pt_a = ptrans(p_a, 1)
        v_a = vproj(O_WVA, O_XA, 1)

        v_sb = [v_v, v_a]
        pt_all = [[pt_v[0], pt_a[0]], [pt_v[1], pt_a[1]]]

        # PV: o[128ch(2 heads), 64q] accumulated per head (col-tiled for h=1)
        o_ps = ps_o.tile([128, 64], f32, tag="o")
        for h in range(2):
            hs = slice(64 * h, 64 * h + 64)
            n = 0
            for side in range(2):
                for t in range(2):
                    nc.tensor.matmul(
                        o_ps[hs, :],
                        v_sb[side][:, t, hs],
                        pt_all[h][side][:, 64 * t:64 * t + 64],
                        start=(n == 0), stop=(n == 3),
                        tile_position=(0, 64 * h),
                    )
                    n += 1
        o_sb = work.tile([128, 64], f32, tag="o_sb")
        nc.scalar.copy(o_sb, o_ps)

        # output projection partial: [64q, 512]
        f_ps = ps_f.tile([64, 512], f32, tag="f_ps")
        nc.tensor.matmul(f_ps, o_sb, col(O_WPROJ, 512), start=True, stop=True)
        f_sb = work.tile([64, 512], f32, tag="f_sb")
        nc.vector.tensor_copy(f_sb[:, 0:256], f_ps[:, 0:256])
        nc.scalar.copy(f_sb[:, 256:512], f_ps[:, 256:512])
        nc.sync.dma_start(out=out_d[:, :], in_=f_sb)

    nc.finalize()
    return nc


def _km(a):
    """[512, C] K-major -> [128, 4*C] (4 k-tiles side by side)."""
    c = a.shape[1]
    return a.reshape(4, 128, c).transpose(1, 0, 2).reshape(128, 4 * c)


def _shard_inputs(xmm, xa, xv, Wq, Wkv, Wproj):
    """Build the 8 per-core input maps (one packed [128, 5376] tensor each)."""
    in_maps = []
    for core in range(N_CORES):
        b, j = divmod(core, 4)
        r = slice(128 * j, 128 * j + 128)               # head-pair rows in [0,512)
        rv = slice(512 + 128 * j, 512 + 128 * j + 128)  # v rows in Wkv
        pack = np.concatenate(
            [
                _km(Wkv[r, :512].T),        # O_WKV
                _km(xv[b].T),               # O_XV
                _km(Wkv[rv, :512].T),       # O_WVV
                Wproj[:, 128 * j:128 * j + 128].T,  # O_WPROJ
                _km(Wq[r, :].T),            # O_WQ
                _km(xmm[b].T),              # O_XMM
                _km(Wkv[r, 512:].T),        # O_WKA
                _km(xa[b].T),               # O_XA
                _km(Wkv[rv, 512:].T),       # O_WVA
            ],
            axis=1,
        )
        assert pack.shape == (128, PACK_COLS)
        in_maps.append({"packA": np.ascontiguousarray(pack, np.float32)})
    return in_maps


def _get_program():
    if "nc" not in _cached:
        _cached["nc"] = _build_program()
    return _cached["nc"]


def _register_ntff_hook():
    """Best-effort: register the axon NTFF profile hook that the container's
    antenv stub doesn't provide, so run_bass_kernel_spmd(trace=True) can
    measure HW exec time. No-op on failure."""
    try:
        import types

        try:
            from antenv.axon_hooks import get_axon_ntff_profile_hook
            if get_axon_ntff_profile_hook() is not None:
                return
        except ImportError:
            pass
        import antenv
        from trn_agent_boot.trn_boot import _ntff_profile_via_ctypes

        hook = _ntff_profile_via_ctypes("/opt/axon/libaxon_pjrt.so")
        mod = types.ModuleType("antenv.axon_hooks")
        mod._hook = hook
        mod.set_axon_ntff_profile_hook = lambda h: setattr(mod, "_hook", h)
        mod.get_axon_ntff_profile_hook = lambda: mod._hook
        sys.modules["antenv.axon_hooks"] = mod
        antenv.axon_hooks = mod

        # artifact upload has no backing store in this container
        from concourse import bass_utils

        bass_utils.upload_artifacts = lambda tmpdir: tmpdir
    except Exception as e:  # pragma: no cover
        print(f"ntff hook registration failed: {e}", file=sys.stderr)


def kernel(xmm, xa, xv, Wq, Wkv, Wproj, bproj, _want_profile=False):
    from concourse.bass_utils import run_bass_kernel_spmd

    if _want_profile:
        _register_ntff_hook()
    nc = _get_program()
    in_maps = _shard_inputs(
        np.asarray(xmm, np.float32), np.asarray(xa, np.float32),
        np.asarray(xv, np.float32), np.asarray(Wq, np.float32),
        np.asarray(Wkv, np.float32), np.asarray(Wproj, np.float32),
    )
    res = run_bass_kernel_spmd(
        nc, in_maps, core_ids=list(range(N_CORES)), trace=_want_profile
    )
    out = np.zeros((B, N_MM, DIM), np.float32)
    for core in range(N_CORES):
        out[core // 4] += res.results[core]["out"]
    out += np.asarray(bproj, np.float32)[None, None, :]
    if _want_profile:
        return out, res
    return out
